# revision 11
# baseline (speedup 1.0000x reference)
"""DirGCNConv on 8 Trainium2 NeuronCores (Bass/Tile).

out = 0.5*(A_norm @ x) @ W_sd.T + 0.5*(A_norm.T @ x) @ W_ds.T + 0.5*(b_sd+b_ds)
with A_norm[r,c] = out_deg(r)^-1/2 * in_deg(c)^-1/2 for each edge (r,c).

Strategy (1D node partition, dest-sharded):
- nodes split into 8 shards of 6250 dests; core p computes out rows of shard p
- x is replicated in each core's HBM as two fp16 tables (rows 0..24999 /
  25000..49999, because dma_gather indices are int16)
- per core, per direction, edges are bucketed by (dest block of 128, source
  half); within a (dir, block-group, half) RUN the per-block slot ranges are
  packed back-to-back (lengths = cross-core max, unrounded) and only the run
  total is padded to a 128 multiple, so 128-edge gather tiles may straddle a
  block boundary; such tiles get one matmul per block they touch.
- gathers are spread round-robin over 4 SWDGE queues
- per (tile, block) the selection matrix S[e, d] = w_e * (d == doff_e) is
  PRECOMPUTED ON HOST and streamed from HBM (one dma_start per (dir, bg)
  unit), so DVE/Act do no per-tile work and the only per-edge on-chip cost
  is the SWDGE gather itself.
  PE accumulates matmul(psum_b, lhsT=M_tile, rhs=S) -> psum[f, d] per block.
  aggT layout [f, 6250] feeds the final linear directly.
- final: per 128-dest chunk, psum[d, fo] = aggT_sd[:,chunk].T @ (0.5 W_sd.T)
  + aggT_ds[:,chunk].T @ (0.5 W_ds.T); add bias; DMA to out.

The program is SPMD-uniform: slot ranges per (dir, half, block) are the max
over cores, padded with (idx=0, no-S-entry) slots.
"""
import os
import sys
import types

sys.path.insert(0, "/opt/trn_rl_repo")
sys.path.insert(0, "/root/.axon_site")

import numpy as np

N = 50000
E = 625000
D = 128
NCORES = 8
SHARD = N // NCORES            # 6250
NBLK = (SHARD + 127) // 128    # 49
HALF = 25000
ALPHA = 0.5

GT = os.environ.get("KERNEL_GT", "float16")   # gather-table / matmul dtype
G_BLOCKS = int(os.environ.get("KERNEL_GBLK", "2"))  # dest blocks per group
GMAX_TILES = int(os.environ.get("KERNEL_GMAX", "15"))  # tiles per gather call
GMAX_LAST = int(os.environ.get("KERNEL_GMAXL", "8"))   # tail-unit chunking
NQUEUES = int(os.environ.get("KERNEL_NQ", "4"))      # SWDGE queues (1..4)
SINGLE_PACKET = bool(int(os.environ.get("KERNEL_SP", "0")))
GATBUFS = int(os.environ.get("KERNEL_GATBUFS", "12"))
SBUFS = int(os.environ.get("KERNEL_SBUFS", "4"))

LAST_EXEC_NS = None


def _np_gt():
    return {"float32": np.float32, "float16": np.float16}[GT]


def _install_ntff_hook():
    try:
        import trn_agent_boot.trn_boot as tb
        mod = types.ModuleType("antenv.axon_hooks")
        _hook = [tb._ntff_profile_via_ctypes('/opt/axon/libaxon_pjrt.so')]
        mod.set_axon_ntff_profile_hook = lambda h: _hook.__setitem__(0, h)
        mod.get_axon_ntff_profile_hook = lambda: _hook[0]
        sys.modules["antenv.axon_hooks"] = mod
        return True
    except Exception:
        return False


def _split_excess_waits(nc, mybir, keep=1):
    """Move excess sync waits onto preceding same-engine NoOps (walrus only
    accepts a limited number of sync-wait commands per instruction)."""
    import bass_rust
    k = 0
    for fn in nc.m.functions:
        for bb in fn.blocks:
            out = []
            changed = False
            for inst in bb.instructions:
                si = inst.sync_info
                waits = list(si.on_wait) if si is not None else []
                if len(waits) > keep:
                    changed = True
                    excess, last = waits[:-keep], waits[-keep:]
                    for w in excess:
                        nop = mybir.InstNoOp(
                            name=f"waitnop-{k}", ins=[], outs=[], engine=inst.engine
                        )
                        k += 1
                        nop.sync_info = bass_rust.SyncInfo(on_wait=[w], on_update=[])
                        nc.register_instruction(nop, overwrite=True)
                        out.append(nop)
                    inst.sync_info = bass_rust.SyncInfo(
                        on_wait=last, on_update=list(si.on_update)
                    )
                out.append(inst)
            if changed:
                bb.instructions = out
    return k


def _plan_and_pack(edge_index, w):
    """Host-side edge partition with run-level packing.

    Returns (plan, idx_all, s_writes_all):
      idx_all[p]: packed int16 index array [128, T_total*8]
      s_writes_all[p]: (srow, scol, sval) writes into the S stream
    """
    row, col = edge_index[0].astype(np.int64), edge_index[1].astype(np.int64)

    # per (dir, core): local-dest-sorted edge arrays
    per = {}
    for di, (dst, src) in enumerate(((row, col), (col, row))):
        shard_of = dst // SHARD
        order = np.argsort(dst, kind="stable")
        dsts, srcs, ws_, sh = dst[order], src[order], w[order], shard_of[order]
        starts = np.searchsorted(sh, np.arange(NCORES + 1))
        for p in range(NCORES):
            s, e = starts[p], starts[p + 1]
            per[(di, p)] = (dsts[s:e] - p * SHARD, srcs[s:e], ws_[s:e])

    # cell edge lists: cells[(dir, half, blk)][core] = (doff, src_local, w)
    cells = {}
    for (di, p), (dl, sl, wl) in per.items():
        blk = dl // 128
        half = (sl >= HALF).astype(np.int64)
        key = blk * 2 + half
        order = np.argsort(key, kind="stable")
        dl, sl, wl, key = dl[order], sl[order], wl[order], key[order]
        bounds = np.searchsorted(key, np.arange(2 * NBLK + 1))
        for b in range(NBLK):
            for h in (0, 1):
                s, e = bounds[b * 2 + h], bounds[b * 2 + h + 1]
                cells.setdefault((di, h, b), {})[p] = (
                    (dl[s:e] - b * 128).astype(np.int64),
                    (sl[s:e] - h * HALF).astype(np.int64),
                    wl[s:e].astype(np.float32),
                )

    # uniform slot lengths (cross-core max, unrounded)
    L = {}
    for (di, h, b), by_core in cells.items():
        L[(di, h, b)] = max(len(v[0]) for v in by_core.values())
    for di in (0, 1):
        for b in range(NBLK):
            if L[(di, 0, b)] + L[(di, 1, b)] == 0:
                L[(di, 0, b)] = 1  # ensure every block has >=1 matmul

    bgs = [list(range(i, min(i + G_BLOCKS, NBLK))) for i in range(0, NBLK, G_BLOCKS)]

    # canonical enumeration: dir -> bg -> half(run) -> packed slots.
    groups = []        # gather calls: dict(dir, bg, half, t0, ntiles)
    unit_of = {}       # (di, gi) -> (tile0, ntiles, m0, n_m)
    slot0 = {}         # (di, h, b) -> absolute slot of cell start
    m_of = {}          # (tile, di, b) -> S-matrix index
    mm_list = {}       # (di, gi) -> [(tile, m, block), ...] emission order
    t_abs = 0
    m_abs = 0
    for di in (0, 1):
        for gi, bg in enumerate(bgs):
            unit_t0, unit_m0 = t_abs, m_abs
            mms = []
            for h in (0, 1):
                run_slot0 = t_abs * 128
                cur = run_slot0
                spans = []  # (block, lo, hi) absolute slot spans
                for b in bg:
                    ln = L[(di, h, b)]
                    slot0[(di, h, b)] = cur
                    if ln > 0:
                        spans.append((b, cur, cur + ln))
                    cur += ln
                run_tiles = (cur - run_slot0 + 127) // 128
                for k in range(run_tiles):
                    tl = run_slot0 + k * 128
                    th = tl + 128
                    for (b, lo, hi) in spans:
                        if lo < th and hi > tl:
                            m_of[(t_abs + k, di, b)] = m_abs
                            mms.append((t_abs + k, m_abs, b))
                            m_abs += 1
                gmax = GMAX_LAST if gi == len(bgs) - 1 else GMAX_TILES
                o = 0
                while o < run_tiles:
                    take = min(gmax, run_tiles - o)
                    groups.append(dict(dir=di, bg=gi, half=h,
                                       t0=t_abs + o, ntiles=take))
                    o += take
                t_abs += run_tiles
            unit_of[(di, gi)] = (unit_t0, t_abs - unit_t0, unit_m0,
                                 m_abs - unit_m0)
            mm_list[(di, gi)] = mms
    T_total = t_abs
    NS_total = m_abs

    # start/stop flags per (di, block) chain, in emission order
    chain = {}
    for di in (0, 1):
        for gi in range(len(bgs)):
            for (t, m, b) in mm_list[(di, gi)]:
                chain.setdefault((di, b), []).append(m)
    flags = {}
    for (di, b), ms in chain.items():
        for i, m in enumerate(ms):
            flags[m] = (i == 0, i == len(ms) - 1)

    # per-core packed idx + S writes
    idx_all, s_writes_all = [], []
    for p in range(NCORES):
        idx16 = np.zeros((T_total * 128,), np.int16)
        rows_l, cols_l, vals_l = [], [], []
        for di in (0, 1):
            for h in (0, 1):
                for b in range(NBLK):
                    dl, sl, wl = cells[(di, h, b)][p]
                    n = len(dl)
                    if n == 0:
                        continue
                    o = slot0[(di, h, b)]
                    idx16[o:o + n] = sl.astype(np.int16)
                    slots = o + np.arange(n)
                    t_arr = slots // 128
                    srow = slots % 128
                    tlo, thi = int(t_arr[0]), int(t_arr[-1])
                    m_per_tile = np.array(
                        [m_of[(t, di, b)] for t in range(tlo, thi + 1)],
                        dtype=np.int64,
                    )
                    m_arr = m_per_tile[t_arr - tlo]
                    rows_l.append(srow)
                    cols_l.append(m_arr * 128 + dl)
                    vals_l.append(wl)
        idx_p = np.tile(idx16.reshape(-1, 16).T, (8, 1)).copy()
        idx_all.append(idx_p)
        s_writes_all.append((np.concatenate(rows_l), np.concatenate(cols_l),
                             np.concatenate(vals_l)))

    # queue assignment: greedy least-loaded (by idx count) in program order
    qload = [0] * NQUEUES
    for g in groups:
        q = min(range(NQUEUES), key=lambda i: qload[i])
        g["queue"] = q
        qload[q] += g["ntiles"] * 128

    plan = dict(bgs=bgs, groups=groups, unit_of=unit_of, mm_list=mm_list,
                flags=flags, T_total=T_total, NS_total=NS_total)
    return plan, idx_all, s_writes_all


def _build_program(plan):
    from concourse import bacc, tile, mybir

    dt_gt = {"float32": mybir.dt.float32, "float16": mybir.dt.float16}[GT]
    bgs, groups, unit_of, mm_list, flags, T_total, NS_total = (
        plan["bgs"], plan["groups"], plan["unit_of"], plan["mm_list"],
        plan["flags"], plan["T_total"], plan["NS_total"],
    )

    nc = bacc.Bacc(None, target_bir_lowering=False, debug=False,
                   num_swdge_queues=NQUEUES)

    t_xlo = nc.declare_dram_parameter("xlo", [HALF, D], dt_gt, isOutput=False)
    t_xhi = nc.declare_dram_parameter("xhi", [HALF, D], dt_gt, isOutput=False)
    t_idx = nc.declare_dram_parameter("idx", [128, T_total * 8], mybir.dt.int16,
                                      isOutput=False)
    t_S = nc.declare_dram_parameter("S", [128, NS_total * 128], dt_gt,
                                    isOutput=False)
    CF_W = 3 * D + 128
    t_cf = nc.declare_dram_parameter("cf32", [128, CF_W], mybir.dt.float32,
                                     isOutput=False)
    t_out = nc.declare_dram_parameter("out", [SHARD, D], mybir.dt.float32,
                                      isOutput=True)

    # idx staging: lead tile covers the first unit of each dir so the first
    # gathers don't wait on the full idx load.
    u00_t0, u00_nt = unit_of[(0, 0)][0], unit_of[(0, 0)][1]
    u10_t0, u10_nt = unit_of[(1, 0)][0], unit_of[(1, 0)][1]

    with tile.TileContext(nc) as tc:
        with (
            tc.tile_pool(name="const", bufs=1) as constp,
            tc.tile_pool(name="agg", bufs=6) as aggp,
            tc.tile_pool(name="gat", bufs=GATBUFS) as gatp,
            tc.tile_pool(name="s", bufs=SBUFS) as sp,
            tc.tile_pool(name="outp", bufs=8) as outp,
            tc.tile_pool(name="psum", bufs=(6 if G_BLOCKS >= 3 else 2 * G_BLOCKS),
                         space="PSUM") as psump,
            tc.tile_pool(name="psumo", bufs=(1 if G_BLOCKS >= 3 else 2),
                         space="PSUM") as psumop,
            tc.tile_pool(name="psumj", bufs=1, space="PSUM") as psumjp,
        ):
            # lead idx tiles (first unit per dir) as separate const tiles
            idxA = constp.tile([128, u00_nt * 8], mybir.dt.int16, tag="idxA")
            idxB = constp.tile([128, u10_nt * 8], mybir.dt.int16, tag="idxB")
            idx_sb = constp.tile([128, T_total * 8], mybir.dt.int16, tag="idx")
            cf_sb = constp.tile([128, CF_W], mybir.dt.float32, tag="cf")
            nc.sync.dma_start(out=idxA[:], in_=t_idx[:, u00_t0 * 8:(u00_t0 + u00_nt) * 8])
            nc.sync.dma_start(out=idxB[:], in_=t_idx[:, u10_t0 * 8:(u10_t0 + u10_nt) * 8])

            def idx_slice(t0, nt):
                # use lead tiles when the range falls inside a lead unit
                if u00_t0 <= t0 and t0 + nt <= u00_t0 + u00_nt:
                    o = t0 - u00_t0
                    return idxA[:, o * 8:(o + nt) * 8]
                if u10_t0 <= t0 and t0 + nt <= u10_t0 + u10_nt:
                    o = t0 - u10_t0
                    return idxB[:, o * 8:(o + nt) * 8]
                return idx_sb[:, t0 * 8:(t0 + nt) * 8]

            by_key = {}
            for g in groups:
                by_key.setdefault((g["dir"], g["bg"]), []).append(g)

            # post the first unit's gathers before the bulk const loads so the
            # SWDGE pipeline starts as early as possible
            n_gather = 0
            gtiles_of = {}
            for di in (0, 1):
                gtiles = []
                for g in by_key[(di, 0)]:
                    t0, nt = g["t0"], g["ntiles"]
                    gt_t = gatp.tile([128, nt, D], dt_gt, tag="g",
                                     name=f"g_lead{di}_{t0}")
                    src = t_xlo if g["half"] == 0 else t_xhi
                    n = nt * 128
                    nc.gpsimd.dma_gather(
                        gt_t[:], src[:], idx_slice(t0, nt),
                        n, n, D, single_packet=SINGLE_PACKET,
                        queue_num=n_gather % NQUEUES,
                    )
                    n_gather += 1
                    gtiles.append([gt_t, t0, nt])
                gtiles_of[di] = gtiles

            # bulk const loads (sync queue) + S streams ride the scalar queue
            nc.sync.dma_start(out=cf_sb[:], in_=t_cf[:])
            nc.sync.dma_start(out=idx_sb[:, 0:T_total * 4], in_=t_idx[:, 0:T_total * 4])
            nc.sync.dma_start(out=idx_sb[:, T_total * 4:], in_=t_idx[:, T_total * 4:])

            w1_sb = cf_sb[:, 0:D]
            w2_sb = cf_sb[:, D:2 * D]
            bias_sb = cf_sb[:, 2 * D:3 * D]
            ones_sb = cf_sb[:, 3 * D:3 * D + 128]

            psum_junk = psumjp.tile([1, 2], mybir.dt.float32, tag="pj")
            # PE observes the const DMA lanes
            nc.tensor.matmul(psum_junk[:1, 0:1], cf_sb[:, 0:1], cf_sb[:, 0:1])

            for gi, bg in enumerate(bgs):
                aggT = [None, None]
                for di in (0, 1):
                    if gi == 0:
                        gtiles = gtiles_of[di]
                    else:
                        gtiles = []
                        for g in by_key[(di, gi)]:
                            t0, nt = g["t0"], g["ntiles"]
                            gt_t = gatp.tile([128, nt, D], dt_gt, tag="g",
                                             name=f"g_{di}_{t0}")
                            src = t_xlo if g["half"] == 0 else t_xhi
                            n = nt * 128
                            nc.gpsimd.dma_gather(
                                gt_t[:], src[:], idx_slice(t0, nt),
                                n, n, D, single_packet=SINGLE_PACKET,
                                queue_num=n_gather % NQUEUES,
                            )
                            n_gather += 1
                            gtiles.append([gt_t, t0, nt])

                    # S stream for the whole (dir, bg) unit (scalar HW queue)
                    ut0, unt, um0, unm = unit_of[(di, gi)]
                    s_t = sp.tile([128, unm, 128], dt_gt, tag="s",
                                  name=f"s_{di}_{gi}")
                    nc.scalar.dma_start(
                        out=s_t[:], in_=t_S[:, um0 * 128:(um0 + unm) * 128]
                    )

                    agg_t = aggp.tile([128, len(bg) * 128], mybir.dt.float32,
                                      tag="agg", name=f"agg_{di}_{gi}")
                    aggT[di] = agg_t

                    # matmuls in tile order; per-block psums
                    psums = {}
                    for (tg, m, b) in mm_list[(di, gi)]:
                        for ge in gtiles:
                            if ge[1] <= tg < ge[1] + ge[2]:
                                gt_t, loc = ge[0], tg - ge[1]
                                break
                        else:
                            raise AssertionError("tile not found")
                        if b not in psums:
                            psums[b] = psump.tile([128, 128], mybir.dt.float32,
                                                  tag="ps", name=f"ps_{di}_{b}")
                        st, sp_ = flags[m]
                        nc.tensor.matmul(
                            psums[b][:], gt_t[:, loc, :], s_t[:, m - um0, :],
                            start=st, stop=sp_,
                        )
                        if sp_:
                            bl = b - bg[0]
                            wc = min(128, SHARD - b * 128)
                            nc.vector.tensor_copy(
                                agg_t[:, bl * 128:bl * 128 + wc],
                                psums[b][:, :wc],
                            )

                # final linear for this block group (both dirs done)
                for b in bg:
                    bl = b - bg[0]
                    c0 = b * 128
                    cl = bl * 128
                    wc = min(128, SHARD - c0)
                    pso = psumop.tile([128, D], mybir.dt.float32, tag="po",
                                      name=f"po_{b}")
                    nc.tensor.matmul(pso[:wc, :], ones_sb[0:1, :wc],
                                     bias_sb[0:1, :], start=True, stop=False)
                    nc.tensor.matmul(pso[:wc, :], aggT[0][:, cl:cl + wc], w1_sb[:],
                                     start=False, stop=False)
                    nc.tensor.matmul(pso[:wc, :], aggT[1][:, cl:cl + wc], w2_sb[:],
                                     start=False, stop=True)
                    o_t = outp.tile([128, D], mybir.dt.float32, tag="o",
                                    name=f"o_{b}")
                    nc.vector.tensor_copy(o_t[:wc, :], pso[:wc, :])
                    nc.sync.dma_start(out=t_out[c0:c0 + wc, :], in_=o_t[:wc, :])

    nc.compile()
    nsplit = _split_excess_waits(nc, __import__("concourse.mybir", fromlist=["x"]))
    if os.environ.get("KERNEL_VERBOSE"):
        print(f"[kernel] split {nsplit} excess waits; T_total={T_total}, "
              f"NS={NS_total}, groups={len(groups)}")
    return nc


def _prepare(x, edge_index, W_sd, b_sd, W_ds, b_ds):
    """Host preprocessing + program build. Returns (nc, in_maps)."""
    x = np.asarray(x, np.float32)
    edge_index = np.asarray(edge_index, np.int32)
    W_sd = np.asarray(W_sd, np.float32)
    b_sd = np.asarray(b_sd, np.float32)
    W_ds = np.asarray(W_ds, np.float32)
    b_ds = np.asarray(b_ds, np.float32)

    # ---- degrees / edge weights (host) ----
    row, col = edge_index[0].astype(np.int64), edge_index[1].astype(np.int64)
    out_deg = np.bincount(row, minlength=N).astype(np.float32)
    in_deg = np.bincount(col, minlength=N).astype(np.float32)
    out_inv = np.where(out_deg > 0, 1.0 / np.sqrt(np.maximum(out_deg, 1)), 0.0)
    in_inv = np.where(in_deg > 0, 1.0 / np.sqrt(np.maximum(in_deg, 1)), 0.0)
    w = (out_inv[row] * in_inv[col]).astype(np.float32)

    plan, idx_all, s_writes_all = _plan_and_pack(edge_index, w)
    NS_total = plan["NS_total"]

    npgt = _np_gt()
    xlo = np.ascontiguousarray(x[:HALF]).astype(npgt)
    xhi = np.ascontiguousarray(x[HALF:]).astype(npgt)
    w1 = (ALPHA * W_sd.T).astype(np.float32).copy()
    w2 = ((1.0 - ALPHA) * W_ds.T).astype(np.float32).copy()
    bias = (ALPHA * b_sd + (1.0 - ALPHA) * b_ds).astype(np.float32)
    bias_bc = np.tile(bias, (128, 1)).copy()
    ones128 = np.ones((128, 128), dtype=np.float32)
    cf32 = np.concatenate([w1, w2, bias_bc, ones128], axis=1).astype(np.float32)

    nc = _build_program(plan)

    in_maps = []
    for p in range(NCORES):
        S = np.zeros((128, NS_total * 128), dtype=npgt)
        srow, scol, sval = s_writes_all[p]
        S[srow, scol] = sval.astype(npgt)
        in_maps.append({
            "xlo": xlo, "xhi": xhi,
            "idx": idx_all[p],
            "S": S,
            "cf32": cf32,
        })
    return nc, in_maps


def kernel(x, edge_index, W_sd, b_sd, W_ds, b_ds):
    global LAST_EXEC_NS
    nc, in_maps = _prepare(x, edge_index, W_sd, b_sd, W_ds, b_ds)

    from concourse.bass_utils import run_bass_kernel_spmd

    want_trace = bool(os.environ.get("KERNEL_TRACE"))
    if want_trace:
        want_trace = _install_ntff_hook()
    core_ids = list(range(NCORES))
    res = run_bass_kernel_spmd(nc, in_maps, core_ids, trace=want_trace)
    LAST_EXEC_NS = res.exec_time_ns

    out = np.concatenate([res.results[p]["out"] for p in range(NCORES)], axis=0)
    return out.astype(np.float32)


# revision 12
# speedup vs baseline: 1.3514x; 1.3514x over previous
"""DirGCNConv on 8 Trainium2 NeuronCores (Bass/Tile).

out = 0.5*(A_norm @ x) @ W_sd.T + 0.5*(A_norm.T @ x) @ W_ds.T + 0.5*(b_sd+b_ds)
with A_norm[r,c] = out_deg(r)^-1/2 * in_deg(c)^-1/2 for each edge (r,c).

Strategy (1D node partition, dest-sharded):
- nodes split into 8 shards of 6250 dests; core p computes out rows of shard p
- x is replicated in each core's HBM as two fp16 tables (rows 0..24999 /
  25000..49999, because dma_gather indices are int16)
- per core, per direction, edges are bucketed by (dest block of 128, source
  half); within a (dir, block-group, half) RUN the per-block slot ranges are
  packed back-to-back (lengths = cross-core max, unrounded) and only the run
  total is padded to a 128 multiple, so 128-edge gather tiles may straddle a
  block boundary; such tiles get one matmul per block they touch.
- gathers are spread round-robin over 4 SWDGE queues
- per (tile, block) the selection matrix S[e, d] = w_e * (d == doff_e) is
  PRECOMPUTED ON HOST and streamed from HBM (one dma_start per (dir, bg)
  unit), so DVE/Act do no per-tile work and the only per-edge on-chip cost
  is the SWDGE gather itself.
  PE accumulates matmul(psum_b, lhsT=M_tile, rhs=S) -> psum[f, d] per block.
  aggT layout [f, 6250] feeds the final linear directly.
- final: per 128-dest chunk, psum[d, fo] = aggT_sd[:,chunk].T @ (0.5 W_sd.T)
  + aggT_ds[:,chunk].T @ (0.5 W_ds.T); add bias; DMA to out.

The program is SPMD-uniform: slot ranges per (dir, half, block) are the max
over cores, padded with (idx=0, no-S-entry) slots.
"""
import os
import sys
import types

sys.path.insert(0, "/opt/trn_rl_repo")
sys.path.insert(0, "/root/.axon_site")

import numpy as np

N = 50000
E = 625000
D = 128
NCORES = 8
SHARD = N // NCORES            # 6250
NBLK = (SHARD + 127) // 128    # 49
HALF = 25000
ALPHA = 0.5

GT = os.environ.get("KERNEL_GT", "float16")   # gather-table / matmul dtype
G_BLOCKS = int(os.environ.get("KERNEL_GBLK", "2"))  # dest blocks per group
GMAX_TILES = int(os.environ.get("KERNEL_GMAX", "15"))  # tiles per gather call
GMAX_LAST = int(os.environ.get("KERNEL_GMAXL", "8"))   # tail-unit chunking
NQUEUES = int(os.environ.get("KERNEL_NQ", "4"))      # SWDGE queues (1..4)
SINGLE_PACKET = bool(int(os.environ.get("KERNEL_SP", "0")))
GATBUFS = int(os.environ.get("KERNEL_GATBUFS", "12"))
SBUFS = int(os.environ.get("KERNEL_SBUFS", "4"))

LAST_EXEC_NS = None


def _np_gt():
    return {"float32": np.float32, "float16": np.float16}[GT]


def _install_ntff_hook():
    try:
        import trn_agent_boot.trn_boot as tb
        mod = types.ModuleType("antenv.axon_hooks")
        _hook = [tb._ntff_profile_via_ctypes('/opt/axon/libaxon_pjrt.so')]
        mod.set_axon_ntff_profile_hook = lambda h: _hook.__setitem__(0, h)
        mod.get_axon_ntff_profile_hook = lambda: _hook[0]
        sys.modules["antenv.axon_hooks"] = mod
        return True
    except Exception:
        return False


def _split_excess_waits(nc, mybir, keep=1):
    """Move excess sync waits onto preceding same-engine NoOps (walrus only
    accepts a limited number of sync-wait commands per instruction)."""
    import bass_rust
    k = 0
    for fn in nc.m.functions:
        for bb in fn.blocks:
            out = []
            changed = False
            for inst in bb.instructions:
                si = inst.sync_info
                waits = list(si.on_wait) if si is not None else []
                if len(waits) > keep:
                    changed = True
                    excess, last = waits[:-keep], waits[-keep:]
                    for w in excess:
                        nop = mybir.InstNoOp(
                            name=f"waitnop-{k}", ins=[], outs=[], engine=inst.engine
                        )
                        k += 1
                        nop.sync_info = bass_rust.SyncInfo(on_wait=[w], on_update=[])
                        nc.register_instruction(nop, overwrite=True)
                        out.append(nop)
                    inst.sync_info = bass_rust.SyncInfo(
                        on_wait=last, on_update=list(si.on_update)
                    )
                out.append(inst)
            if changed:
                bb.instructions = out
    return k


def _plan_and_pack(edge_index, w):
    """Host-side edge partition with run-level packing.

    Returns (plan, idx_all, s_writes_all):
      idx_all[p]: packed int16 index array [128, T_total*8]
      s_writes_all[p]: (srow, scol, sval) writes into the S stream
    """
    row, col = edge_index[0].astype(np.int64), edge_index[1].astype(np.int64)

    # per (dir, core): local-dest-sorted edge arrays
    per = {}
    for di, (dst, src) in enumerate(((row, col), (col, row))):
        shard_of = dst // SHARD
        order = np.argsort(dst, kind="stable")
        dsts, srcs, ws_, sh = dst[order], src[order], w[order], shard_of[order]
        starts = np.searchsorted(sh, np.arange(NCORES + 1))
        for p in range(NCORES):
            s, e = starts[p], starts[p + 1]
            per[(di, p)] = (dsts[s:e] - p * SHARD, srcs[s:e], ws_[s:e])

    # cell edge lists: cells[(dir, half, blk)][core] = (doff, src_local, w)
    cells = {}
    for (di, p), (dl, sl, wl) in per.items():
        blk = dl // 128
        half = (sl >= HALF).astype(np.int64)
        key = blk * 2 + half
        order = np.argsort(key, kind="stable")
        dl, sl, wl, key = dl[order], sl[order], wl[order], key[order]
        bounds = np.searchsorted(key, np.arange(2 * NBLK + 1))
        for b in range(NBLK):
            for h in (0, 1):
                s, e = bounds[b * 2 + h], bounds[b * 2 + h + 1]
                cells.setdefault((di, h, b), {})[p] = (
                    (dl[s:e] - b * 128).astype(np.int64),
                    (sl[s:e] - h * HALF).astype(np.int64),
                    wl[s:e].astype(np.float32),
                )

    # uniform slot lengths (cross-core max, unrounded)
    L = {}
    for (di, h, b), by_core in cells.items():
        L[(di, h, b)] = max(len(v[0]) for v in by_core.values())
    for di in (0, 1):
        for b in range(NBLK):
            if L[(di, 0, b)] + L[(di, 1, b)] == 0:
                L[(di, 0, b)] = 1  # ensure every block has >=1 matmul

    bgs = [list(range(i, min(i + G_BLOCKS, NBLK))) for i in range(0, NBLK, G_BLOCKS)]

    # canonical enumeration: dir -> bg -> half(run) -> packed slots.
    groups = []        # gather calls: dict(dir, bg, half, t0, ntiles)
    unit_of = {}       # (di, gi) -> (tile0, ntiles, m0, n_m)
    slot0 = {}         # (di, h, b) -> absolute slot of cell start
    m_of = {}          # (tile, di, b) -> S-matrix index
    mm_list = {}       # (di, gi) -> [(tile, m, block), ...] emission order
    t_abs = 0
    m_abs = 0
    for di in (0, 1):
        for gi, bg in enumerate(bgs):
            unit_t0, unit_m0 = t_abs, m_abs
            mms = []
            for h in (0, 1):
                run_slot0 = t_abs * 128
                cur = run_slot0
                spans = []  # (block, lo, hi) absolute slot spans
                for b in bg:
                    ln = L[(di, h, b)]
                    slot0[(di, h, b)] = cur
                    if ln > 0:
                        spans.append((b, cur, cur + ln))
                    cur += ln
                run_tiles = (cur - run_slot0 + 127) // 128
                for k in range(run_tiles):
                    tl = run_slot0 + k * 128
                    th = tl + 128
                    for (b, lo, hi) in spans:
                        if lo < th and hi > tl:
                            m_of[(t_abs + k, di, b)] = m_abs
                            mms.append((t_abs + k, m_abs, b))
                            m_abs += 1
                gmax = GMAX_LAST if gi == len(bgs) - 1 else GMAX_TILES
                o = 0
                while o < run_tiles:
                    take = min(gmax, run_tiles - o)
                    groups.append(dict(dir=di, bg=gi, half=h,
                                       t0=t_abs + o, ntiles=take))
                    o += take
                t_abs += run_tiles
            unit_of[(di, gi)] = (unit_t0, t_abs - unit_t0, unit_m0,
                                 m_abs - unit_m0)
            mm_list[(di, gi)] = mms
    T_total = t_abs
    NS_total = m_abs

    # start/stop flags per (di, block) chain, in emission order
    chain = {}
    for di in (0, 1):
        for gi in range(len(bgs)):
            for (t, m, b) in mm_list[(di, gi)]:
                chain.setdefault((di, b), []).append(m)
    flags = {}
    for (di, b), ms in chain.items():
        for i, m in enumerate(ms):
            flags[m] = (i == 0, i == len(ms) - 1)

    # per-core packed idx + S writes
    idx_all, s_writes_all = [], []
    for p in range(NCORES):
        idx16 = np.zeros((T_total * 128,), np.int16)
        rows_l, cols_l, vals_l = [], [], []
        for di in (0, 1):
            for h in (0, 1):
                for b in range(NBLK):
                    dl, sl, wl = cells[(di, h, b)][p]
                    n = len(dl)
                    if n == 0:
                        continue
                    o = slot0[(di, h, b)]
                    idx16[o:o + n] = sl.astype(np.int16)
                    slots = o + np.arange(n)
                    t_arr = slots // 128
                    srow = slots % 128
                    tlo, thi = int(t_arr[0]), int(t_arr[-1])
                    m_per_tile = np.array(
                        [m_of[(t, di, b)] for t in range(tlo, thi + 1)],
                        dtype=np.int64,
                    )
                    m_arr = m_per_tile[t_arr - tlo]
                    rows_l.append(srow)
                    cols_l.append(m_arr * 128 + dl)
                    vals_l.append(wl)
        idx_p = np.tile(idx16.reshape(-1, 16).T, (8, 1)).copy()
        idx_all.append(idx_p)
        s_writes_all.append((np.concatenate(rows_l), np.concatenate(cols_l),
                             np.concatenate(vals_l)))

    plan = dict(bgs=bgs, groups=groups, unit_of=unit_of, mm_list=mm_list,
                flags=flags, T_total=T_total, NS_total=NS_total)
    return plan, idx_all, s_writes_all


def _build_program(plan):
    from concourse import bacc, tile, mybir

    dt_gt = {"float32": mybir.dt.float32, "float16": mybir.dt.float16}[GT]
    bgs, groups, unit_of, mm_list, flags, T_total, NS_total = (
        plan["bgs"], plan["groups"], plan["unit_of"], plan["mm_list"],
        plan["flags"], plan["T_total"], plan["NS_total"],
    )

    nc = bacc.Bacc(None, target_bir_lowering=False, debug=False,
                   num_swdge_queues=NQUEUES)

    t_xlo = nc.declare_dram_parameter("xlo", [HALF, D], dt_gt, isOutput=False)
    t_xhi = nc.declare_dram_parameter("xhi", [HALF, D], dt_gt, isOutput=False)
    t_idx = nc.declare_dram_parameter("idx", [128, T_total * 8], mybir.dt.int16,
                                      isOutput=False)
    t_S = nc.declare_dram_parameter("S", [128, NS_total * 128], dt_gt,
                                    isOutput=False)
    CF_W = 3 * D + 128
    t_cf = nc.declare_dram_parameter("cf32", [128, CF_W], mybir.dt.float32,
                                     isOutput=False)
    t_out = nc.declare_dram_parameter("out", [SHARD, D], mybir.dt.float32,
                                      isOutput=True)

    # idx staging: lead tile covers the first unit of each dir so the first
    # gathers don't wait on the full idx load.
    u00_t0, u00_nt = unit_of[(0, 0)][0], unit_of[(0, 0)][1]
    u10_t0, u10_nt = unit_of[(1, 0)][0], unit_of[(1, 0)][1]

    with tile.TileContext(nc) as tc:
        with (
            tc.tile_pool(name="const", bufs=1) as constp,
            tc.tile_pool(name="agg", bufs=6) as aggp,
            tc.tile_pool(name="gat", bufs=GATBUFS) as gatp,
            tc.tile_pool(name="s", bufs=SBUFS) as sp,
            tc.tile_pool(name="outp", bufs=8) as outp,
            tc.tile_pool(name="psum", bufs=(6 if G_BLOCKS >= 3 else 2 * G_BLOCKS),
                         space="PSUM") as psump,
            tc.tile_pool(name="psumo", bufs=(1 if G_BLOCKS >= 3 else 2),
                         space="PSUM") as psumop,
            tc.tile_pool(name="psumj", bufs=1, space="PSUM") as psumjp,
        ):
            # lead idx tiles (first unit per dir) as separate const tiles
            idxA = constp.tile([128, u00_nt * 8], mybir.dt.int16, tag="idxA")
            idxB = constp.tile([128, u10_nt * 8], mybir.dt.int16, tag="idxB")
            idx_sb = constp.tile([128, T_total * 8], mybir.dt.int16, tag="idx")
            cf_sb = constp.tile([128, CF_W], mybir.dt.float32, tag="cf")
            nc.sync.dma_start(out=idxA[:], in_=t_idx[:, u00_t0 * 8:(u00_t0 + u00_nt) * 8])
            nc.sync.dma_start(out=idxB[:], in_=t_idx[:, u10_t0 * 8:(u10_t0 + u10_nt) * 8])

            def idx_slice(t0, nt):
                # use lead tiles when the range falls inside a lead unit
                if u00_t0 <= t0 and t0 + nt <= u00_t0 + u00_nt:
                    o = t0 - u00_t0
                    return idxA[:, o * 8:(o + nt) * 8]
                if u10_t0 <= t0 and t0 + nt <= u10_t0 + u10_nt:
                    o = t0 - u10_t0
                    return idxB[:, o * 8:(o + nt) * 8]
                return idx_sb[:, t0 * 8:(t0 + nt) * 8]

            by_key = {}
            for g in groups:
                by_key.setdefault((g["dir"], g["bg"]), []).append(g)

            # post the first unit's gathers before the bulk const loads so the
            # SWDGE pipeline starts as early as possible
            n_gather = 0
            gtiles_of = {}
            for di in (0, 1):
                gtiles = []
                for g in by_key[(di, 0)]:
                    t0, nt = g["t0"], g["ntiles"]
                    gt_t = gatp.tile([128, nt, D], dt_gt, tag="g",
                                     name=f"g_lead{di}_{t0}")
                    src = t_xlo if g["half"] == 0 else t_xhi
                    n = nt * 128
                    nc.gpsimd.dma_gather(
                        gt_t[:], src[:], idx_slice(t0, nt),
                        n, n, D, single_packet=SINGLE_PACKET,
                        queue_num=n_gather % NQUEUES,
                    )
                    n_gather += 1
                    gtiles.append([gt_t, t0, nt])
                gtiles_of[di] = gtiles

            # bulk const loads (sync queue) + S streams ride the scalar queue
            nc.sync.dma_start(out=cf_sb[:], in_=t_cf[:])
            nc.sync.dma_start(out=idx_sb[:, 0:T_total * 4], in_=t_idx[:, 0:T_total * 4])
            nc.sync.dma_start(out=idx_sb[:, T_total * 4:], in_=t_idx[:, T_total * 4:])

            w1_sb = cf_sb[:, 0:D]
            w2_sb = cf_sb[:, D:2 * D]
            bias_sb = cf_sb[:, 2 * D:3 * D]
            ones_sb = cf_sb[:, 3 * D:3 * D + 128]

            psum_junk = psumjp.tile([1, 2], mybir.dt.float32, tag="pj")
            # PE observes the const DMA lanes
            nc.tensor.matmul(psum_junk[:1, 0:1], cf_sb[:, 0:1], cf_sb[:, 0:1])

            for gi, bg in enumerate(bgs):
                aggT = [None, None]
                for di in (0, 1):
                    if gi == 0:
                        gtiles = gtiles_of[di]
                    else:
                        gtiles = []
                        for g in by_key[(di, gi)]:
                            t0, nt = g["t0"], g["ntiles"]
                            gt_t = gatp.tile([128, nt, D], dt_gt, tag="g",
                                             name=f"g_{di}_{t0}")
                            src = t_xlo if g["half"] == 0 else t_xhi
                            n = nt * 128
                            nc.gpsimd.dma_gather(
                                gt_t[:], src[:], idx_slice(t0, nt),
                                n, n, D, single_packet=SINGLE_PACKET,
                                queue_num=n_gather % NQUEUES,
                            )
                            n_gather += 1
                            gtiles.append([gt_t, t0, nt])

                    # S stream for the whole (dir, bg) unit (scalar HW queue)
                    ut0, unt, um0, unm = unit_of[(di, gi)]
                    s_t = sp.tile([128, unm, 128], dt_gt, tag="s",
                                  name=f"s_{di}_{gi}")
                    nc.scalar.dma_start(
                        out=s_t[:], in_=t_S[:, um0 * 128:(um0 + unm) * 128]
                    )

                    agg_t = aggp.tile([128, len(bg) * 128], mybir.dt.float32,
                                      tag="agg", name=f"agg_{di}_{gi}")
                    aggT[di] = agg_t

                    # matmuls in tile order; per-block psums
                    psums = {}
                    for (tg, m, b) in mm_list[(di, gi)]:
                        for ge in gtiles:
                            if ge[1] <= tg < ge[1] + ge[2]:
                                gt_t, loc = ge[0], tg - ge[1]
                                break
                        else:
                            raise AssertionError("tile not found")
                        if b not in psums:
                            psums[b] = psump.tile([128, 128], mybir.dt.float32,
                                                  tag="ps", name=f"ps_{di}_{b}")
                        st, sp_ = flags[m]
                        nc.tensor.matmul(
                            psums[b][:], gt_t[:, loc, :], s_t[:, m - um0, :],
                            start=st, stop=sp_,
                        )
                        if sp_:
                            bl = b - bg[0]
                            wc = min(128, SHARD - b * 128)
                            nc.vector.tensor_copy(
                                agg_t[:, bl * 128:bl * 128 + wc],
                                psums[b][:, :wc],
                            )

                # final linear for this block group (both dirs done)
                for b in bg:
                    bl = b - bg[0]
                    c0 = b * 128
                    cl = bl * 128
                    wc = min(128, SHARD - c0)
                    pso = psumop.tile([128, D], mybir.dt.float32, tag="po",
                                      name=f"po_{b}")
                    nc.tensor.matmul(pso[:wc, :], ones_sb[0:1, :wc],
                                     bias_sb[0:1, :], start=True, stop=False)
                    nc.tensor.matmul(pso[:wc, :], aggT[0][:, cl:cl + wc], w1_sb[:],
                                     start=False, stop=False)
                    nc.tensor.matmul(pso[:wc, :], aggT[1][:, cl:cl + wc], w2_sb[:],
                                     start=False, stop=True)
                    o_t = outp.tile([128, D], mybir.dt.float32, tag="o",
                                    name=f"o_{b}")
                    nc.vector.tensor_copy(o_t[:wc, :], pso[:wc, :])
                    nc.sync.dma_start(out=t_out[c0:c0 + wc, :], in_=o_t[:wc, :])

    nc.compile()
    nsplit = _split_excess_waits(nc, __import__("concourse.mybir", fromlist=["x"]))
    if os.environ.get("KERNEL_VERBOSE"):
        print(f"[kernel] split {nsplit} excess waits; T_total={T_total}, "
              f"NS={NS_total}, groups={len(groups)}")
    return nc


def _prepare(x, edge_index, W_sd, b_sd, W_ds, b_ds):
    """Host preprocessing + program build. Returns (nc, in_maps)."""
    x = np.asarray(x, np.float32)
    edge_index = np.asarray(edge_index, np.int32)
    W_sd = np.asarray(W_sd, np.float32)
    b_sd = np.asarray(b_sd, np.float32)
    W_ds = np.asarray(W_ds, np.float32)
    b_ds = np.asarray(b_ds, np.float32)

    # ---- degrees / edge weights (host) ----
    row, col = edge_index[0].astype(np.int64), edge_index[1].astype(np.int64)
    out_deg = np.bincount(row, minlength=N).astype(np.float32)
    in_deg = np.bincount(col, minlength=N).astype(np.float32)
    out_inv = np.where(out_deg > 0, 1.0 / np.sqrt(np.maximum(out_deg, 1)), 0.0)
    in_inv = np.where(in_deg > 0, 1.0 / np.sqrt(np.maximum(in_deg, 1)), 0.0)
    w = (out_inv[row] * in_inv[col]).astype(np.float32)

    plan, idx_all, s_writes_all = _plan_and_pack(edge_index, w)
    NS_total = plan["NS_total"]

    npgt = _np_gt()
    xlo = np.ascontiguousarray(x[:HALF]).astype(npgt)
    xhi = np.ascontiguousarray(x[HALF:]).astype(npgt)
    w1 = (ALPHA * W_sd.T).astype(np.float32).copy()
    w2 = ((1.0 - ALPHA) * W_ds.T).astype(np.float32).copy()
    bias = (ALPHA * b_sd + (1.0 - ALPHA) * b_ds).astype(np.float32)
    bias_bc = np.tile(bias, (128, 1)).copy()
    ones128 = np.ones((128, 128), dtype=np.float32)
    cf32 = np.concatenate([w1, w2, bias_bc, ones128], axis=1).astype(np.float32)

    nc = _build_program(plan)

    in_maps = []
    for p in range(NCORES):
        S = np.zeros((128, NS_total * 128), dtype=npgt)
        srow, scol, sval = s_writes_all[p]
        S[srow, scol] = sval.astype(npgt)
        in_maps.append({
            "xlo": xlo, "xhi": xhi,
            "idx": idx_all[p],
            "S": S,
            "cf32": cf32,
        })
    return nc, in_maps


def kernel(x, edge_index, W_sd, b_sd, W_ds, b_ds):
    global LAST_EXEC_NS
    nc, in_maps = _prepare(x, edge_index, W_sd, b_sd, W_ds, b_ds)

    from concourse.bass_utils import run_bass_kernel_spmd

    want_trace = bool(os.environ.get("KERNEL_TRACE"))
    if want_trace:
        want_trace = _install_ntff_hook()
    core_ids = list(range(NCORES))
    res = run_bass_kernel_spmd(nc, in_maps, core_ids, trace=want_trace)
    LAST_EXEC_NS = res.exec_time_ns

    out = np.concatenate([res.results[p]["out"] for p in range(NCORES)], axis=0)
    return out.astype(np.float32)


# revision 16
# speedup vs baseline: 1.3610x; 1.0071x over previous
"""DirGCNConv on 8 Trainium2 NeuronCores (Bass/Tile).

out = 0.5*(A_norm @ x) @ W_sd.T + 0.5*(A_norm.T @ x) @ W_ds.T + 0.5*(b_sd+b_ds)
with A_norm[r,c] = out_deg(r)^-1/2 * in_deg(c)^-1/2 for each edge (r,c).

Strategy (1D node partition, dest-sharded):
- nodes split into 8 shards of 6250 dests; core p computes out rows of shard p
- x is replicated in each core's HBM as two fp16 tables (rows 0..24999 /
  25000..49999, because dma_gather indices are int16)
- per core, per direction, edges are bucketed by (dest block of 128, source
  half); within a (dir, block-group, half) RUN the per-block slot ranges are
  packed back-to-back (lengths = cross-core max, unrounded) and only the run
  total is padded to a 128 multiple, so 128-edge gather tiles may straddle a
  block boundary; such tiles get one matmul per block they touch.
- gathers are spread round-robin over 4 SWDGE queues
- per (tile, block) the selection matrix S[e, d] = w_e * (d == doff_e) is
  PRECOMPUTED ON HOST and streamed from HBM (one dma_start per (dir, bg)
  unit), so DVE/Act do no per-tile work and the only per-edge on-chip cost
  is the SWDGE gather itself.
  PE accumulates matmul(psum_b, lhsT=M_tile, rhs=S) -> psum[f, d] per block.
  aggT layout [f, 6250] feeds the final linear directly.
- final: per 128-dest chunk, psum[d, fo] = aggT_sd[:,chunk].T @ (0.5 W_sd.T)
  + aggT_ds[:,chunk].T @ (0.5 W_ds.T); add bias; DMA to out.

The program is SPMD-uniform: slot ranges per (dir, half, block) are the max
over cores, padded with (idx=0, no-S-entry) slots.
"""
import os
import sys
import types

sys.path.insert(0, "/opt/trn_rl_repo")
sys.path.insert(0, "/root/.axon_site")

import numpy as np

N = 50000
E = 625000
D = 128
NCORES = 8
SHARD = N // NCORES            # 6250
NBLK = (SHARD + 127) // 128    # 49
HALF = 25000
ALPHA = 0.5

GT = os.environ.get("KERNEL_GT", "float16")   # gather-table / matmul dtype
G_BLOCKS = int(os.environ.get("KERNEL_GBLK", "2"))  # dest blocks per group
GMAX_TILES = int(os.environ.get("KERNEL_GMAX", "15"))  # tiles per gather call
GMAX_LAST = int(os.environ.get("KERNEL_GMAXL", "8"))   # tail-unit chunking
NQUEUES = int(os.environ.get("KERNEL_NQ", "4"))      # SWDGE queues (1..4)
SINGLE_PACKET = bool(int(os.environ.get("KERNEL_SP", "0")))
GATBUFS = int(os.environ.get("KERNEL_GATBUFS", "12"))
SBUFS = int(os.environ.get("KERNEL_SBUFS", "4"))

LAST_EXEC_NS = None


def _np_gt():
    return {"float32": np.float32, "float16": np.float16}[GT]


def _install_ntff_hook():
    try:
        import trn_agent_boot.trn_boot as tb
        mod = types.ModuleType("antenv.axon_hooks")
        _hook = [tb._ntff_profile_via_ctypes('/opt/axon/libaxon_pjrt.so')]
        mod.set_axon_ntff_profile_hook = lambda h: _hook.__setitem__(0, h)
        mod.get_axon_ntff_profile_hook = lambda: _hook[0]
        sys.modules["antenv.axon_hooks"] = mod
        return True
    except Exception:
        return False


def _split_excess_waits(nc, mybir, keep=1):
    """Move excess sync waits onto preceding same-engine NoOps (walrus only
    accepts a limited number of sync-wait commands per instruction)."""
    import bass_rust
    k = 0
    for fn in nc.m.functions:
        for bb in fn.blocks:
            out = []
            changed = False
            for inst in bb.instructions:
                si = inst.sync_info
                waits = list(si.on_wait) if si is not None else []
                if len(waits) > keep:
                    changed = True
                    excess, last = waits[:-keep], waits[-keep:]
                    for w in excess:
                        nop = mybir.InstNoOp(
                            name=f"waitnop-{k}", ins=[], outs=[], engine=inst.engine
                        )
                        k += 1
                        nop.sync_info = bass_rust.SyncInfo(on_wait=[w], on_update=[])
                        nc.register_instruction(nop, overwrite=True)
                        out.append(nop)
                    inst.sync_info = bass_rust.SyncInfo(
                        on_wait=last, on_update=list(si.on_update)
                    )
                out.append(inst)
            if changed:
                bb.instructions = out
    return k


def _plan_and_pack(edge_index, w):
    """Host-side edge partition with run-level packing.

    Returns (plan, idx_all, s_writes_all):
      idx_all[p]: packed int16 index array [128, T_total*8]
      s_writes_all[p]: (srow, scol, sval) writes into the S stream
    """
    row, col = edge_index[0].astype(np.int64), edge_index[1].astype(np.int64)

    # per (dir, core): local-dest-sorted edge arrays
    per = {}
    for di, (dst, src) in enumerate(((row, col), (col, row))):
        shard_of = dst // SHARD
        order = np.argsort(dst, kind="stable")
        dsts, srcs, ws_, sh = dst[order], src[order], w[order], shard_of[order]
        starts = np.searchsorted(sh, np.arange(NCORES + 1))
        for p in range(NCORES):
            s, e = starts[p], starts[p + 1]
            per[(di, p)] = (dsts[s:e] - p * SHARD, srcs[s:e], ws_[s:e])

    # cell edge lists: cells[(dir, half, blk)][core] = (doff, src_local, w)
    cells = {}
    for (di, p), (dl, sl, wl) in per.items():
        blk = dl // 128
        half = (sl >= HALF).astype(np.int64)
        key = blk * 2 + half
        order = np.argsort(key, kind="stable")
        dl, sl, wl, key = dl[order], sl[order], wl[order], key[order]
        bounds = np.searchsorted(key, np.arange(2 * NBLK + 1))
        for b in range(NBLK):
            for h in (0, 1):
                s, e = bounds[b * 2 + h], bounds[b * 2 + h + 1]
                cells.setdefault((di, h, b), {})[p] = (
                    (dl[s:e] - b * 128).astype(np.int64),
                    (sl[s:e] - h * HALF).astype(np.int64),
                    wl[s:e].astype(np.float32),
                )

    # uniform slot lengths (cross-core max, unrounded)
    L = {}
    for (di, h, b), by_core in cells.items():
        L[(di, h, b)] = max(len(v[0]) for v in by_core.values())
    for di in (0, 1):
        for b in range(NBLK):
            if L[(di, 0, b)] + L[(di, 1, b)] == 0:
                L[(di, 0, b)] = 1  # ensure every block has >=1 matmul

    bgs = [list(range(i, min(i + G_BLOCKS, NBLK))) for i in range(0, NBLK, G_BLOCKS)]

    # canonical enumeration: dir -> bg -> half(run) -> packed slots.
    groups = []        # gather calls: dict(dir, bg, half, t0, ntiles)
    unit_of = {}       # (di, gi) -> (tile0, ntiles, m0, n_m)
    slot0 = {}         # (di, h, b) -> absolute slot of cell start
    m_of = {}          # (tile, di, b) -> S-matrix index
    mm_list = {}       # (di, gi) -> [(tile, m, block), ...] emission order
    t_abs = 0
    m_abs = 0
    for di in (0, 1):
        for gi, bg in enumerate(bgs):
            unit_t0, unit_m0 = t_abs, m_abs
            mms = []
            for h in (0, 1):
                run_slot0 = t_abs * 128
                cur = run_slot0
                spans = []  # (block, lo, hi) absolute slot spans
                for b in bg:
                    ln = L[(di, h, b)]
                    slot0[(di, h, b)] = cur
                    if ln > 0:
                        spans.append((b, cur, cur + ln))
                    cur += ln
                run_tiles = (cur - run_slot0 + 127) // 128
                for k in range(run_tiles):
                    tl = run_slot0 + k * 128
                    th = tl + 128
                    for (b, lo, hi) in spans:
                        if lo < th and hi > tl:
                            m_of[(t_abs + k, di, b)] = m_abs
                            mms.append((t_abs + k, m_abs, b))
                            m_abs += 1
                gmax = GMAX_LAST if gi == len(bgs) - 1 else GMAX_TILES
                o = 0
                while o < run_tiles:
                    take = min(gmax, run_tiles - o)
                    groups.append(dict(dir=di, bg=gi, half=h,
                                       t0=t_abs + o, ntiles=take))
                    o += take
                t_abs += run_tiles
            unit_of[(di, gi)] = (unit_t0, t_abs - unit_t0, unit_m0,
                                 m_abs - unit_m0)
            mm_list[(di, gi)] = mms
    T_total = t_abs
    NS_total = m_abs

    # start/stop flags per (di, block) chain, in emission order
    chain = {}
    for di in (0, 1):
        for gi in range(len(bgs)):
            for (t, m, b) in mm_list[(di, gi)]:
                chain.setdefault((di, b), []).append(m)
    flags = {}
    for (di, b), ms in chain.items():
        for i, m in enumerate(ms):
            flags[m] = (i == 0, i == len(ms) - 1)

    # per-core packed idx + S writes
    idx_all, s_writes_all = [], []
    for p in range(NCORES):
        idx16 = np.zeros((T_total * 128,), np.int16)
        rows_l, cols_l, vals_l = [], [], []
        for di in (0, 1):
            for h in (0, 1):
                for b in range(NBLK):
                    dl, sl, wl = cells[(di, h, b)][p]
                    n = len(dl)
                    if n == 0:
                        continue
                    o = slot0[(di, h, b)]
                    idx16[o:o + n] = sl.astype(np.int16)
                    slots = o + np.arange(n)
                    t_arr = slots // 128
                    srow = slots % 128
                    tlo, thi = int(t_arr[0]), int(t_arr[-1])
                    m_per_tile = np.array(
                        [m_of[(t, di, b)] for t in range(tlo, thi + 1)],
                        dtype=np.int64,
                    )
                    m_arr = m_per_tile[t_arr - tlo]
                    rows_l.append(srow)
                    cols_l.append(m_arr * 128 + dl)
                    vals_l.append(wl)
        idx_p = np.tile(idx16.reshape(-1, 16).T, (8, 1)).copy()
        idx_all.append(idx_p)
        s_writes_all.append((np.concatenate(rows_l), np.concatenate(cols_l),
                             np.concatenate(vals_l)))

    plan = dict(bgs=bgs, groups=groups, unit_of=unit_of, mm_list=mm_list,
                flags=flags, T_total=T_total, NS_total=NS_total)
    return plan, idx_all, s_writes_all


def _build_program(plan):
    from concourse import bacc, tile, mybir

    dt_gt = {"float32": mybir.dt.float32, "float16": mybir.dt.float16}[GT]
    bgs, groups, unit_of, mm_list, flags, T_total, NS_total = (
        plan["bgs"], plan["groups"], plan["unit_of"], plan["mm_list"],
        plan["flags"], plan["T_total"], plan["NS_total"],
    )

    nc = bacc.Bacc(None, target_bir_lowering=False, debug=False,
                   num_swdge_queues=NQUEUES)

    t_xlo = nc.declare_dram_parameter("xlo", [HALF, D], dt_gt, isOutput=False)
    t_xhi = nc.declare_dram_parameter("xhi", [HALF, D], dt_gt, isOutput=False)
    t_idx = nc.declare_dram_parameter("idx", [128, T_total * 8], mybir.dt.int16,
                                      isOutput=False)
    t_S = nc.declare_dram_parameter("S", [128, NS_total * 128], dt_gt,
                                    isOutput=False)
    # final-linear consts in fp16: fp32 rhs would put PE on its 4x-slower path
    CF_W = 3 * D + 128
    t_cf = nc.declare_dram_parameter("cf32", [128, CF_W], mybir.dt.float16,
                                     isOutput=False)
    t_out = nc.declare_dram_parameter("out", [SHARD, D], mybir.dt.float32,
                                      isOutput=True)

    # idx staging: lead tile covers the first unit of each dir so the first
    # gathers don't wait on the full idx load.
    u00_t0, u00_nt = unit_of[(0, 0)][0], unit_of[(0, 0)][1]
    u10_t0, u10_nt = unit_of[(1, 0)][0], unit_of[(1, 0)][1]

    with tile.TileContext(nc) as tc:
        with (
            tc.tile_pool(name="const", bufs=1) as constp,
            tc.tile_pool(name="agg", bufs=6) as aggp,
            tc.tile_pool(name="gat", bufs=GATBUFS) as gatp,
            tc.tile_pool(name="s", bufs=SBUFS) as sp,
            tc.tile_pool(name="outp", bufs=8) as outp,
            tc.tile_pool(name="psum", bufs=(6 if G_BLOCKS >= 3 else 2 * G_BLOCKS),
                         space="PSUM") as psump,
            tc.tile_pool(name="psumo", bufs=(1 if G_BLOCKS >= 3 else 2),
                         space="PSUM") as psumop,
            tc.tile_pool(name="psumj", bufs=1, space="PSUM") as psumjp,
        ):
            # lead idx tiles (first unit per dir) as separate const tiles
            idxA = constp.tile([128, u00_nt * 8], mybir.dt.int16, tag="idxA")
            idxB = constp.tile([128, u10_nt * 8], mybir.dt.int16, tag="idxB")
            idx_sb = constp.tile([128, T_total * 8], mybir.dt.int16, tag="idx")
            cf_sb = constp.tile([128, CF_W], mybir.dt.float16, tag="cf")
            nc.sync.dma_start(out=idxA[:], in_=t_idx[:, u00_t0 * 8:(u00_t0 + u00_nt) * 8])
            nc.sync.dma_start(out=idxB[:], in_=t_idx[:, u10_t0 * 8:(u10_t0 + u10_nt) * 8])

            def idx_slice(t0, nt):
                # use lead tiles when the range falls inside a lead unit
                if u00_t0 <= t0 and t0 + nt <= u00_t0 + u00_nt:
                    o = t0 - u00_t0
                    return idxA[:, o * 8:(o + nt) * 8]
                if u10_t0 <= t0 and t0 + nt <= u10_t0 + u10_nt:
                    o = t0 - u10_t0
                    return idxB[:, o * 8:(o + nt) * 8]
                return idx_sb[:, t0 * 8:(t0 + nt) * 8]

            by_key = {}
            for g in groups:
                by_key.setdefault((g["dir"], g["bg"]), []).append(g)

            # post the first unit's gathers before the bulk const loads so the
            # SWDGE pipeline starts as early as possible
            n_gather = 0
            gtiles_of = {}
            for di in (0, 1):
                gtiles = []
                for g in by_key[(di, 0)]:
                    t0, nt = g["t0"], g["ntiles"]
                    gt_t = gatp.tile([128, nt, D], dt_gt, tag="g",
                                     name=f"g_lead{di}_{t0}")
                    src = t_xlo if g["half"] == 0 else t_xhi
                    n = nt * 128
                    nc.gpsimd.dma_gather(
                        gt_t[:], src[:], idx_slice(t0, nt),
                        n, n, D, single_packet=SINGLE_PACKET,
                        queue_num=n_gather % NQUEUES,
                    )
                    n_gather += 1
                    gtiles.append([gt_t, t0, nt])
                gtiles_of[di] = gtiles

            # bulk const loads (sync queue) + S streams ride the scalar queue
            nc.sync.dma_start(out=cf_sb[:], in_=t_cf[:])
            nc.sync.dma_start(out=idx_sb[:, 0:T_total * 4], in_=t_idx[:, 0:T_total * 4])
            nc.sync.dma_start(out=idx_sb[:, T_total * 4:], in_=t_idx[:, T_total * 4:])

            w1_sb = cf_sb[:, 0:D]
            w2_sb = cf_sb[:, D:2 * D]
            bias_sb = cf_sb[:, 2 * D:3 * D]
            ones_sb = cf_sb[:, 3 * D:3 * D + 128]

            psum_junk = psumjp.tile([1, 2], mybir.dt.float32, tag="pj")
            # PE observes the const DMA lanes
            nc.tensor.matmul(psum_junk[:1, 0:1], cf_sb[:, 0:1], cf_sb[:, 0:1])

            for gi, bg in enumerate(bgs):
                aggT = [None, None]
                for di in (0, 1):
                    if gi == 0:
                        gtiles = gtiles_of[di]
                    else:
                        gtiles = []
                        for g in by_key[(di, gi)]:
                            t0, nt = g["t0"], g["ntiles"]
                            gt_t = gatp.tile([128, nt, D], dt_gt, tag="g",
                                             name=f"g_{di}_{t0}")
                            src = t_xlo if g["half"] == 0 else t_xhi
                            n = nt * 128
                            nc.gpsimd.dma_gather(
                                gt_t[:], src[:], idx_slice(t0, nt),
                                n, n, D, single_packet=SINGLE_PACKET,
                                queue_num=n_gather % NQUEUES,
                            )
                            n_gather += 1
                            gtiles.append([gt_t, t0, nt])

                    # S stream for the whole (dir, bg) unit (scalar HW queue)
                    ut0, unt, um0, unm = unit_of[(di, gi)]
                    s_t = sp.tile([128, unm, 128], dt_gt, tag="s",
                                  name=f"s_{di}_{gi}")
                    nc.scalar.dma_start(
                        out=s_t[:], in_=t_S[:, um0 * 128:(um0 + unm) * 128]
                    )

                    agg_t = aggp.tile([128, len(bg) * 128], mybir.dt.float16,
                                      tag="agg", name=f"agg_{di}_{gi}")
                    aggT[di] = agg_t

                    # matmuls in tile order; per-block psums
                    psums = {}
                    for (tg, m, b) in mm_list[(di, gi)]:
                        for ge in gtiles:
                            if ge[1] <= tg < ge[1] + ge[2]:
                                gt_t, loc = ge[0], tg - ge[1]
                                break
                        else:
                            raise AssertionError("tile not found")
                        if b not in psums:
                            psums[b] = psump.tile([128, 128], mybir.dt.float32,
                                                  tag="ps", name=f"ps_{di}_{b}")
                        st, sp_ = flags[m]
                        nc.tensor.matmul(
                            psums[b][:], gt_t[:, loc, :], s_t[:, m - um0, :],
                            start=st, stop=sp_,
                        )
                        if sp_:
                            bl = b - bg[0]
                            wc = min(128, SHARD - b * 128)
                            nc.vector.tensor_copy(
                                agg_t[:, bl * 128:bl * 128 + wc],
                                psums[b][:, :wc],
                            )

                # final linear for this block group (both dirs done)
                for b in bg:
                    bl = b - bg[0]
                    c0 = b * 128
                    cl = bl * 128
                    wc = min(128, SHARD - c0)
                    pso = psumop.tile([128, D], mybir.dt.float32, tag="po",
                                      name=f"po_{b}")
                    nc.tensor.matmul(pso[:wc, :], ones_sb[0:1, :wc],
                                     bias_sb[0:1, :], start=True, stop=False)
                    nc.tensor.matmul(pso[:wc, :], aggT[0][:, cl:cl + wc], w1_sb[:],
                                     start=False, stop=False)
                    nc.tensor.matmul(pso[:wc, :], aggT[1][:, cl:cl + wc], w2_sb[:],
                                     start=False, stop=True)
                    o_t = outp.tile([128, D], mybir.dt.float32, tag="o",
                                    name=f"o_{b}")
                    nc.vector.tensor_copy(o_t[:wc, :], pso[:wc, :])
                    nc.sync.dma_start(out=t_out[c0:c0 + wc, :], in_=o_t[:wc, :])

    nc.compile()
    nsplit = _split_excess_waits(nc, __import__("concourse.mybir", fromlist=["x"]))
    if os.environ.get("KERNEL_VERBOSE"):
        print(f"[kernel] split {nsplit} excess waits; T_total={T_total}, "
              f"NS={NS_total}, groups={len(groups)}")
    return nc


def _prepare(x, edge_index, W_sd, b_sd, W_ds, b_ds):
    """Host preprocessing + program build. Returns (nc, in_maps)."""
    x = np.asarray(x, np.float32)
    edge_index = np.asarray(edge_index, np.int32)
    W_sd = np.asarray(W_sd, np.float32)
    b_sd = np.asarray(b_sd, np.float32)
    W_ds = np.asarray(W_ds, np.float32)
    b_ds = np.asarray(b_ds, np.float32)

    # ---- degrees / edge weights (host) ----
    row, col = edge_index[0].astype(np.int64), edge_index[1].astype(np.int64)
    out_deg = np.bincount(row, minlength=N).astype(np.float32)
    in_deg = np.bincount(col, minlength=N).astype(np.float32)
    out_inv = np.where(out_deg > 0, 1.0 / np.sqrt(np.maximum(out_deg, 1)), 0.0)
    in_inv = np.where(in_deg > 0, 1.0 / np.sqrt(np.maximum(in_deg, 1)), 0.0)
    w = (out_inv[row] * in_inv[col]).astype(np.float32)

    plan, idx_all, s_writes_all = _plan_and_pack(edge_index, w)
    NS_total = plan["NS_total"]

    npgt = _np_gt()
    xlo = np.ascontiguousarray(x[:HALF]).astype(npgt)
    xhi = np.ascontiguousarray(x[HALF:]).astype(npgt)
    w1 = (ALPHA * W_sd.T).astype(np.float32).copy()
    w2 = ((1.0 - ALPHA) * W_ds.T).astype(np.float32).copy()
    bias = (ALPHA * b_sd + (1.0 - ALPHA) * b_ds).astype(np.float32)
    bias_bc = np.tile(bias, (128, 1)).copy()
    ones128 = np.ones((128, 128), dtype=np.float32)
    cf32 = np.concatenate([w1, w2, bias_bc, ones128], axis=1).astype(np.float16)

    nc = _build_program(plan)

    in_maps = []
    for p in range(NCORES):
        S = np.zeros((128, NS_total * 128), dtype=npgt)
        srow, scol, sval = s_writes_all[p]
        S[srow, scol] = sval.astype(npgt)
        in_maps.append({
            "xlo": xlo, "xhi": xhi,
            "idx": idx_all[p],
            "S": S,
            "cf32": cf32,
        })
    return nc, in_maps


def kernel(x, edge_index, W_sd, b_sd, W_ds, b_ds):
    global LAST_EXEC_NS
    nc, in_maps = _prepare(x, edge_index, W_sd, b_sd, W_ds, b_ds)

    from concourse.bass_utils import run_bass_kernel_spmd

    want_trace = bool(os.environ.get("KERNEL_TRACE"))
    if want_trace:
        want_trace = _install_ntff_hook()
    core_ids = list(range(NCORES))
    res = run_bass_kernel_spmd(nc, in_maps, core_ids, trace=want_trace)
    LAST_EXEC_NS = res.exec_time_ns

    out = np.concatenate([res.results[p]["out"] for p in range(NCORES)], axis=0)
    return out.astype(np.float32)
